# revision 20
# baseline (speedup 1.0000x reference)
"""MultiHeadCrossAttention Trainium2 kernel (8-core SPMD, query-parallel).

Sharding: core c handles batch b=c//4, query rows [1024*(c%4), +1024), all 8
heads.  Each core returns a disjoint [256, 1024] slice of out^T for its batch;
the host gather is a pure concat + transpose.

On-device layout is fully transposed ([channel, position]), matching the raw
[B, C, H, W] input layout, so no transposes are needed anywhere:
  q^T/k^T : [d, pos]   via  lhsT=W^T chunk [c,32|128], rhs=x^T chunk [c, pos]
  scores^T: [kpos, q]  3 key-chunks per PSUM tile [128,1536] (3 banks)
  exp     : ACT, PSUM->SBUF bf16, FD=1536 (the kernel's critical path:
            176 ACTIVATEs x ~1.4us; ~245us ACT-busy floor)
  attn@v  : lhsT=[v|1] [128,33], rhs=p^T [128,512]; 2x col-tiled into ONE
            PSUM bank: group A partitions 0-33 (tile_position (0,0)), group
            B partitions 64-97 ((0,64)), same columns; the ones column
            yields softmax denominators in rows 32/96
  norm    : per-head denominators -> approx-reciprocal [16,64]; gpsimd
            partition-broadcast to a base-0 [32,NQ] strip, DMA-hop into a
            4-head-packed [128,NQ] tile; one DVE mul+add per 4-head group
  final   : per 4-head group: 4 row-tiled concurrent matmuls accumulate in
            one PSUM bank; 1 DVE copy/add [128,512] per (dc,qb) into yacc

Emission order software-pipelines head h+1's projections under head h's
attention so the ACT engine starts exp'ing within ~10us of kernel start.
All DVE/ACT ops keep in/out on identical partition ranges (walrus verifier
requirement); every cross-partition move rides on DMA or the PE.
"""

import numpy as np
import ml_dtypes

B, C, N, HEADS, D = 2, 256, 4096, 8, 32
NQ = 1024          # queries per core
NCORES = 8
CC = C // 128      # contraction chunks (2)

BF16 = ml_dtypes.bfloat16

_cached = {}
# assist_tiles: per-(h,qb) tile indices whose exp runs on the Vector engine
# via the custom polynomial ops below instead of the Scalar engine, to push
# past the ACT-only exp throughput floor
CFG = {"debug": False, "assist_tiles": (), "col_av": False}

# ---- DVE-assist exp: p = P(u)^2 ~ e^s with u = ALPHA*s (ALPHA folded into
# Wq host-side).  P = (z^3 + E2 z^2 + E1 z + E0) + u(O2 z^2 + O1 z + O0),
# z = u^2 — a monic-normalized deg-6 fit of e^(s/2) on s in [-8.2, 8.2]
# (actual score range [-7.7, 7.9]); measured per-row softmax weight error
# 0.8% mean / 1.1% max if applied to ALL elements.  Two chained custom DVE
# ops (6 + 8 ALU stages); P^2 >= 0 so weights can never go negative.
EXP_ALPHA = 0.18748615654951978
EXP_E = (0.99158194, 3.81915894, 1.34754594)   # e0, e1, e2
EXP_O = (2.69978975, 2.76711082, 1.75416515)   # o0, o1, o2


def _register_exp_ops():
    import numpy as np
    from concourse import dve_ops
    from concourse.dve_spec import Spec, Src0, Src1, C0, C1, C2, sq

    if "EXP_EVEN_ANT" in dve_ops._SUB_OPCODE_FOR_NAME:
        return

    def _ref_even(in0, in1, c0, c1, c2):
        z = in0.astype(np.float32) * in0.astype(np.float32)
        return (((z + c0) * z + c1) * z + c2).astype(np.float32)

    def _ref_odd(in0, in1, c0, c1, c2):
        i0 = in0.astype(np.float32)
        z = i0 * i0
        p = ((z * c0 + c1) * z + c2) * i0 + in1.astype(np.float32)
        return (p * p).astype(np.float32)

    z = Src0 * Src0
    even = dve_ops.DveOp(
        "EXP_EVEN_ANT",
        Spec(body=((z + C0) * z + C1) * z + C2, reference=_ref_even),
        subdim=False,
        uops_sha={"v3": "995c3409d8b17014", "v4": "6b354a64fe8e5ebc"},
    )
    odd = dve_ops.DveOp(
        "EXP_ODD_SQ_ANT",
        Spec(body=sq(((z * C0 + C1) * z + C2) * Src0 + Src1),
             reference=_ref_odd),
        subdim=False,
        uops_sha={"v3": "3ca347f746f9ce9c", "v4": "3a667804bd745fc2"},
    )
    for op in (even, odd):
        dve_ops.OPS.append(op)
        dve_ops.CUSTOM_DVE_SPECS[op.name] = op.spec
        dve_ops._SUB_OPCODE_FOR_NAME[op.name] = (
            max(dve_ops._SUB_OPCODE_FOR_NAME.values()) + 1)
    return

# chunk emission order: flattened (m, g) order of the original kernel;
# consecutive chunks cycle strips g=0,1,2,3 so any 3 consecutive chunks hit
# distinct PE row-groups (concurrent matmuls).  kc = 16*(m//4) + 4g + (m%4).
CHUNK_ORDER = [16 * (m // 4) + 4 * g + (m % 4) for m in range(8) for g in range(4)]
# tiles of 3 chunks (last tile has 2)
TILES = [CHUNK_ORDER[i:i + 3] for i in range(0, 32, 3)]
N_TILES = len(TILES)  # 11


def _build_nc():
    import concourse.bass as bass
    import concourse.bacc as bacc
    import concourse.tile as tile
    import concourse.mybir as mybir
    from contextlib import ExitStack

    fp32 = mybir.dt.float32
    bf16 = mybir.dt.bfloat16
    Exp = mybir.ActivationFunctionType.Exp

    assist = tuple(CFG["assist_tiles"])
    exp_even = exp_odd = None
    if assist:
        _register_exp_ops()
        from concourse import dve_ops
        by_name = {op.name: op for op in dve_ops.OPS}
        exp_even, exp_odd = by_name["EXP_EVEN_ANT"], by_name["EXP_ODD_SQ_ANT"]
    # with assist, PSUM scores hold u = ALPHA*s (ALPHA folded into Wq);
    # the ACT path then rescales by 1/ALPHA via the free affine
    act_scale = (1.0 / EXP_ALPHA) if assist else 1.0

    nc = bacc.Bacc("TRN2", target_bir_lowering=False, debug=False,
                   num_devices=NCORES)

    src_d = nc.dram_tensor("src_bf", [C, N], bf16, kind="ExternalInput")
    tgt_d = nc.dram_tensor("tgt_bf", [C, NQ], bf16, kind="ExternalInput")
    tgt8_d = nc.dram_tensor("tgt8", [128, 2 * NQ], fp32, kind="ExternalInput")
    wq4_d = nc.dram_tensor("wq4", [C, HEADS * 128], bf16, kind="ExternalInput")
    wk_d = nc.dram_tensor("wkT", [C, C], bf16, kind="ExternalInput")
    wv_d = nc.dram_tensor("wvT", [C, C], bf16, kind="ExternalInput")
    wo8_d = nc.dram_tensor("wo8", [128, 2 * C], bf16, kind="ExternalInput")
    y_d = nc.dram_tensor("yT", [C, NQ], fp32, kind="ExternalOutput")

    with tile.TileContext(nc) as tc, ExitStack() as ctx:
        konst = ctx.enter_context(tc.tile_pool(name="konst", bufs=1))
        work = ctx.enter_context(tc.tile_pool(name="work", bufs=1))
        p_pool = ctx.enter_context(tc.tile_pool(name="p", bufs=3))
        sm_pool = ctx.enter_context(tc.tile_pool(name="sm", bufs=2))
        xb_pool = ctx.enter_context(tc.tile_pool(name="xb", bufs=2))
        # PSUM budget (8 banks): ps tiles [128,1536] (3 banks) x2 bufs = 6,
        # po/pj tiles [128,512] (1 bank) x2 bufs = 2
        ps_pool = ctx.enter_context(tc.tile_pool(name="ps", bufs=2, space="PSUM"))
        po_pool = ctx.enter_context(tc.tile_pool(name="po", bufs=2, space="PSUM"))
        pj_pool = po_pool
        texp_pool = (ctx.enter_context(tc.tile_pool(name="texp", bufs=2))
                     if assist else None)

        # ---- input tiles ---------------------------------------------------
        src_sb = konst.tile([128, CC * N], bf16, tag="src")
        tgt_sb = konst.tile([128, CC * NQ], bf16, tag="tgt")
        tgt8_sb = konst.tile([128, 2 * NQ], fp32, tag="tgt8")
        wq4_sb = konst.tile([128, CC * HEADS * 128], bf16, tag="wq4")
        wk_sb = konst.tile([128, CC * C], bf16, tag="wk")
        wv_sb = konst.tile([128, CC * C], bf16, tag="wv")
        wo8_sb = konst.tile([128, 2 * C], bf16, tag="wo8")

        def dma_w(w_sb, w_d):
            for cc in range(CC):
                nc.sync.dma_start(w_sb[:, cc * C:(cc + 1) * C],
                                  w_d.ap()[128 * cc:128 * (cc + 1), :])

        def dma_src_half(half):
            for cc in range(CC):
                nc.sync.dma_start(
                    src_sb[:, cc * N + 2048 * half: cc * N + 2048 * (half + 1)],
                    src_d.ap()[128 * cc:128 * (cc + 1),
                               2048 * half:2048 * (half + 1)])

        # DMA order: everything head-0's first tiles need lands first.
        dma_w(wk_sb, wk_d)
        dma_w(wv_sb, wv_d)
        for cc in range(CC):
            nc.sync.dma_start(tgt_sb[:, cc * NQ:(cc + 1) * NQ],
                              tgt_d.ap()[128 * cc:128 * (cc + 1), :])
        for cc in range(CC):
            nc.sync.dma_start(wq4_sb[:, cc * 1024:(cc + 1) * 1024],
                              wq4_d.ap()[128 * cc:128 * (cc + 1), :])
        dma_src_half(0)
        dma_src_half(1)
        nc.sync.dma_start(tgt8_sb[:], tgt8_d.ap()[:, :])
        nc.sync.dma_start(wo8_sb[:], wo8_d.ap()[:, :])

        # ---- persistent tiles ---------------------------------------------
        kT = [konst.tile([128, 1024], bf16, tag=f"kT{h}", name=f"kT{h}")
              for h in range(HEADS)]
        qT = [konst.tile([128, NQ], bf16, tag=f"qT{h}", name=f"qT{h}")
              for h in range(HEADS)]
        v_sb = konst.tile([128, HEADS * 33 * 32], bf16, tag="v")
        for h in range(HEADS):
            ones_ap = v_sb[:].rearrange("p (h k c) -> p h k c", h=HEADS, k=32)[
                :, h, :, 32:33]
            nc.gpsimd.memset(ones_ap, 1.0)
        # 4-head-packed weighted sums: group g holds heads 4g..4g+3 at
        # partition strips 32*(h%4)
        xw4 = [work.tile([128, NQ], fp32, tag=f"xw4_{g}", name=f"xw4_{g}")
               for g in range(2)]
        rbs4 = [work.tile([128, NQ], fp32, tag=f"rbs4_{g}", name=f"rbs4_{g}")
                for g in range(2)]
        # per-head raw denominator rows, col-group A and B separately so the
        # A+B add keeps identical partition ranges (walrus rule); rows
        # 8*qb + r cover queries 512*qb + 64*r .. +64
        su_a = [work.tile([16, 64], fp32, tag=f"sua{h}", name=f"sua{h}")
                for h in range(HEADS)]
        su_b = [work.tile([16, 64], fp32, tag=f"sub{h}", name=f"sub{h}")
                for h in range(HEADS)]
        ssum = [work.tile([16, 64], fp32, tag=f"ss{h}", name=f"ss{h}")
                for h in range(HEADS)]
        rsum = [work.tile([16, 64], fp32, tag=f"rs{h}", name=f"rs{h}")
                for h in range(HEADS)]
        yacc = [work.tile([128, NQ], fp32, tag=f"yacc{t}", name=f"yacc{t}")
                for t in range(CC)]

        v_done = set()

        def vproj(kc):
            if kc in v_done:
                return
            v_done.add(kc)
            ps = pj_pool.tile([128, 512], fp32, tag="po", name=f"psv{kc}")
            for cc in range(CC):
                nc.tensor.matmul(
                    ps[:, 0:256],
                    lhsT=src_sb[:, cc * N + 128 * kc: cc * N + 128 * kc + 128],
                    rhs=wv_sb[:, cc * C:(cc + 1) * C],
                    start=(cc == 0), stop=(cc == CC - 1),
                    tile_position=(0, 0))
            dest = v_sb[:].rearrange("p (h k c) -> p h k c", h=HEADS, k=32)[
                :, :, kc, 0:32]
            nc.vector.tensor_copy(dest, ps[:, 0:256])

        def kqproj_steps(h):
            # k^T folded: strip g (partitions 32g..) holds kpos block b=4jj+g
            # at cols [512jj, +512); kc for 128-col slice m: 16*(m//4)+4g+(m%4)
            steps = []

            def k_step(jj):
                def run():
                    ps = pj_pool.tile([128, 512], fp32, tag="po",
                                      name=f"psk{h}_{jj}")
                    # g outer / cc inner: each col-group's accumulation group
                    # closes before the next opens (same PSUM bank)
                    for g in range(4):
                        for cc in range(CC):
                            blk = 4 * jj + g
                            nc.tensor.matmul(
                                ps[32 * g:32 * g + 32, 0:512],
                                lhsT=wk_sb[:, cc * C + 32 * h: cc * C + 32 * h + 32],
                                rhs=src_sb[:, cc * N + 512 * blk: cc * N + 512 * blk + 512],
                                start=(cc == 0), stop=(cc == CC - 1),
                                tile_position=(0, 32 * g))
                    nc.vector.tensor_copy(
                        kT[h][:, 512 * jj:512 * jj + 512], ps[:, 0:512])
                return run

            def q_step(qb):
                def run():
                    ps = pj_pool.tile([128, 512], fp32, tag="po",
                                      name=f"psq{h}_{qb}")
                    for cc in range(CC):
                        nc.tensor.matmul(
                            ps[:, 0:512],
                            lhsT=wq4_sb[:, cc * 1024 + 128 * h: cc * 1024 + 128 * h + 128],
                            rhs=tgt_sb[:, cc * NQ + 512 * qb: cc * NQ + 512 * qb + 512],
                            start=(cc == 0), stop=(cc == CC - 1),
                            tile_position=(0, 0))
                    nc.vector.tensor_copy(qT[h][:, 512 * qb:512 * qb + 512],
                                          ps[:, 0:512])
                return run

            steps.append(k_step(0))
            steps.append(q_step(0))
            steps.append(q_step(1))
            steps.append(k_step(1))
            return steps

        def attn_unit(h, qb, feed=(), feed_at=4):
            """One (head, 512-query-block) attention unit: 11 PSUM tiles of
            3 key-chunks (last: 2); exp at FD=1536; attn@v col-tiled 2x into
            one PSUM bank (A: partitions 0-33, B: 64-97)."""
            feed = list(feed)
            po = po_pool.tile([128, 512], fp32, tag="po", name=f"po{h}_{qb}")
            pos = 0  # global chunk position (0..31) for A/B parity + flags
            for ti, chunks in enumerate(TILES):
                nchunk = len(chunks)
                if h == 0 and qb == 0:
                    for kc in chunks:
                        vproj(kc)
                if feed and ti >= feed_at:
                    feed.pop(0)()
                ps = ps_pool.tile([128, 1536], fp32, tag="ps",
                                  name=f"ps{h}_{qb}_{ti}")
                for j, kc in enumerate(chunks):
                    g, m = (kc % 16) // 4, 4 * (kc // 16) + (kc % 4)
                    nc.tensor.matmul(
                        ps[:, 512 * j:512 * j + 512],
                        lhsT=kT[h][32 * g:32 * g + 32, 128 * m:128 * m + 128],
                        rhs=qT[h][32 * g:32 * g + 32, 512 * qb:512 * qb + 512],
                        start=True, stop=True,
                        tile_position=(32 * g, 0))
                p_sb = p_pool.tile([128, 1536], bf16, tag="p",
                                   name=f"p{h}_{qb}_{ti}")
                if ti in assist:
                    t1 = texp_pool.tile([128, 1536], fp32, tag="t1",
                                        name=f"t1_{h}_{qb}_{ti}")
                    nc.vector._custom_dve(
                        exp_even, out=t1[:, 0:512 * nchunk],
                        in0=ps[:, 0:512 * nchunk],
                        s0=EXP_E[2], s1=EXP_E[1], imm2=EXP_E[0])
                    nc.vector._custom_dve(
                        exp_odd, out=p_sb[:, 0:512 * nchunk],
                        in0=ps[:, 0:512 * nchunk],
                        in1=t1[:, 0:512 * nchunk],
                        s0=EXP_O[2], s1=EXP_O[1], imm2=EXP_O[0])
                else:
                    nc.scalar.activation(p_sb[:, 0:512 * nchunk],
                                         ps[:, 0:512 * nchunk], Exp,
                                         scale=act_scale)
                for j, kc in enumerate(chunks):
                    gpos = pos + j
                    if CFG["col_av"]:
                        # A/B groups pend concurrently in one bank at disjoint
                        # partitions; HW has_written is per-partition — skip
                        # the sim's coarse zero-region check
                        co = 0 if gpos % 2 == 0 else 64
                        st, sp = gpos < 2, gpos >= 30
                    else:
                        co = 0
                        st, sp = gpos == 0, gpos == 31
                    nc.tensor.matmul(
                        po[co:co + 33, 0:512],
                        lhsT=v_sb[:, 1056 * h + 33 * kc: 1056 * h + 33 * kc + 33],
                        rhs=p_sb[:, 512 * j:512 * j + 512],
                        start=st, stop=sp,
                        tile_position=(0, co), skip_group_check=True)
                pos += nchunk
            for st in feed:
                st()
            # drain: copy the weighted-sum rows to SBUF, DMA into the 4-head-
            # packed xw4; denominator row(s) -> per-head su tiles via DMA
            g4, j4 = h // 4, h % 4
            tA = xb_pool.tile([32, 512], fp32, tag="tA", name=f"tA{h}{qb}")
            nc.vector.tensor_copy(tA[:], po[0:32, 0:512])
            stmp = sm_pool.tile([97, 512], fp32, tag="stmp", name=f"st{h}{qb}")
            nc.vector.tensor_copy(stmp[32:33, 0:512], po[32:33, 0:512])
            nc.sync.dma_start(su_a[h][8 * qb:8 * qb + 8, 0:64],
                              stmp[32:33, 0:512])
            if CFG["col_av"]:
                nc.vector.tensor_copy(stmp[96:97, 0:512], po[96:97, 0:512])
                nc.sync.dma_start(su_b[h][8 * qb:8 * qb + 8, 0:64],
                                  stmp[96:97, 0:512])
                xb64 = xb_pool.tile([96, 512], fp32, tag="xb64",
                                    name=f"xb64_{h}{qb}")
                nc.vector.tensor_copy(xb64[64:96, 0:512], po[64:96, 0:512])
                xb0 = xb_pool.tile([32, 512], fp32, tag="xb0",
                                   name=f"xb0_{h}{qb}")
                nc.sync.dma_start(xb0[:], xb64[64:96, 0:512])
                nc.vector.tensor_add(tA[:], tA[:], xb0[:])
            nc.sync.dma_start(
                xw4[g4][32 * j4:32 * j4 + 32, 512 * qb:512 * qb + 512], tA[:])

        def recip_bcast(h):
            """Denominators for head h -> 1/d broadcast into rbs4 strip."""
            if CFG["col_av"]:
                nc.vector.tensor_add(ssum[h][:], su_a[h][:], su_b[h][:])
                nc.vector.reciprocal_approx_fast(rsum[h][:], ssum[h][:])
            else:
                nc.vector.reciprocal_approx_fast(rsum[h][:], su_a[h][:])
            rrow = sm_pool.tile([1, NQ], fp32, tag="rrow", name=f"rr{h}")
            nc.sync.dma_start(rrow[:], rsum[h][:])
            rbs = sm_pool.tile([32, NQ], fp32, tag="rbs", name=f"rb{h}")
            nc.gpsimd.partition_broadcast(rbs[:], rrow[:])
            g4, j4 = h // 4, h % 4
            nc.sync.dma_start(rbs4[g4][32 * j4:32 * j4 + 32, :], rbs[:])

        def normalize_group(g4):
            """Scale 4 heads' weighted sums, add residual, project, accum."""
            nc.vector.tensor_mul(xw4[g4][:], xw4[g4][:], rbs4[g4][:])
            xfh = xb_pool.tile([128, NQ], bf16, tag="xfh", name=f"xf{g4}")
            nc.vector.tensor_add(xfh[:], xw4[g4][:],
                                 tgt8_sb[:, NQ * g4:NQ * (g4 + 1)])
            for dc in range(CC):
                for qb in range(NQ // 512):
                    ps = pj_pool.tile([128, 512], fp32, tag="po",
                                      name=f"py{g4}_{dc}_{qb}")
                    # 4-head sum = plain K=128 contraction over the packed
                    # partition layout: one full-array matmul
                    nc.tensor.matmul(
                        ps[:, 0:512],
                        lhsT=wo8_sb[:, C * g4 + 128 * dc: C * g4 + 128 * dc + 128],
                        rhs=xfh[:, 512 * qb:512 * qb + 512],
                        start=True, stop=True, tile_position=(0, 0))
                    if g4 == 0:
                        nc.vector.tensor_copy(
                            yacc[dc][:, 512 * qb:512 * qb + 512], ps[:, 0:512])
                    else:
                        nc.vector.tensor_add(
                            yacc[dc][:, 512 * qb:512 * qb + 512],
                            yacc[dc][:, 512 * qb:512 * qb + 512], ps[:, 0:512])
                if g4 == 1:
                    nc.sync.dma_start(y_d.ap()[128 * dc:128 * (dc + 1), :],
                                      yacc[dc][:])

        # ---- emission: software-pipeline projections under attention ------
        s0 = kqproj_steps(0)
        s0[0]()  # k_step(0)
        s0[1]()  # q_step(0)
        s0[2]()  # q_step(1)
        for h in range(HEADS):
            steps = kqproj_steps(h + 1) if h + 1 < HEADS else []
            if h == 0:
                # k_step(1) of head 0 feeds at tile 4 (chunks 0-14 only need
                # jj=0 cols; src half 1 lands ~11us in)
                attn_unit(h, 0, feed=[s0[3]] + steps[:1], feed_at=4)
                attn_unit(h, 1, feed=steps[1:])
            else:
                attn_unit(h, 0, feed=steps)
                if h == 4:
                    normalize_group(0)
                attn_unit(h, 1)
            # per-head reciprocal+broadcast as soon as both qb done; the
            # gpsimd/DMA latency hides under head h+1's attention
            recip_bcast(h)
        normalize_group(1)

    nc.compile()
    return nc


def _prep_core_inputs(core, tgt, src, Wq, Wk, Wv, Wo):
    b, qoff = core // 4, NQ * (core % 4)
    srcT = src[b].reshape(C, N)
    tgtT = tgt[b].reshape(C, N)[:, qoff:qoff + NQ]
    scale = 1.0 / np.sqrt(np.float32(D))
    if CFG["assist_tiles"]:
        scale = scale * np.float32(EXP_ALPHA)
    wqT = (Wq * scale).T.astype(BF16)
    wq4 = np.empty((C, HEADS * 128), dtype=BF16)
    for h in range(HEADS):
        wq4[:, 128 * h:128 * (h + 1)] = np.tile(wqT[:, 32 * h:32 * h + 32],
                                                (1, 4))
    # 4-head-packed residual / Wo blocks: group g, head h=4g+j at partition
    # strip 32j
    tgt8 = np.empty((128, 2 * NQ), dtype=np.float32)
    woT = Wo.T.astype(np.float32)
    wo8 = np.empty((128, 2 * C), dtype=BF16)
    for g in range(2):
        for j in range(4):
            h = 4 * g + j
            tgt8[32 * j:32 * (j + 1), NQ * g:NQ * (g + 1)] = \
                tgtT[32 * h:32 * h + 32, :]
            wo8[32 * j:32 * (j + 1), C * g:C * (g + 1)] = \
                woT[32 * h:32 * h + 32, :].astype(BF16)
    return {
        "src_bf": np.ascontiguousarray(srcT).astype(BF16),
        "tgt_bf": np.ascontiguousarray(tgtT).astype(BF16),
        "tgt8": tgt8,
        "wq4": wq4,
        "wkT": np.ascontiguousarray(Wk.T).astype(BF16),
        "wvT": np.ascontiguousarray(Wv.T).astype(BF16),
        "wo8": wo8,
    }


def kernel(tgt, src, Wq, Wk, Wv, Wo, _want_results=False):
    from concourse.bass_utils import run_bass_kernel_spmd

    tgt = np.asarray(tgt, dtype=np.float32)
    src = np.asarray(src, dtype=np.float32)
    Wq = np.asarray(Wq, dtype=np.float32)
    Wk = np.asarray(Wk, dtype=np.float32)
    Wv = np.asarray(Wv, dtype=np.float32)
    Wo = np.asarray(Wo, dtype=np.float32)

    if "nc" not in _cached:
        _cached["nc"] = _build_nc()
    nc = _cached["nc"]

    in_maps = [_prep_core_inputs(c, tgt, src, Wq, Wk, Wv, Wo)
               for c in range(NCORES)]
    res = run_bass_kernel_spmd(nc, in_maps, core_ids=list(range(NCORES)))

    out = np.empty((B, N, C), dtype=np.float32)
    for c in range(NCORES):
        b, qoff = c // 4, NQ * (c % 4)
        out[b, qoff:qoff + NQ, :] = res.results[c]["yT"].T
    if _want_results:
        return out, res
    return out
